# revision 80
# baseline (speedup 1.0000x reference)
"""Trainium2 Bass kernel for nn_EnhancedConformationalConsistencyLoss.

Strategy (segment_reduce, 8 cores):
  - Host: stable-sort nodes by fragment id; fragments f in [32*d, 32*(d+1))
    go to device d, then bin-packed (first-fit decreasing) into 128-node
    partition blocks so no fragment crosses a block boundary; 9 blocks
    (NCAP=1152) suffice. O(N) index prep + O(NF) final combine only; all
    O(N*H^2)/O(pairs) work is on-device.
  - Device (per core, SPMD with per-core shards), two phases so PE streams
    continuously ("front" = MLP for all 4 feature matrices, "back" =
    normalize + pair terms):
      * MLP feature-major, weights stationary: h^T = W1'.T @ [short^T;long^T]
        with the short/long lerp folded into the K=512-stacked scaled W1';
        softplus as Ln(Exp(x+b1)+1) (both + Square live in one ACT table set,
        table thrash patched out); si^T = W2.T @ sp, b2' (carrying the
        ShiftedSoftplus -log2) added during the PSUM drain as a per-partition
        tensor_scalar.
      * si node-major via DMA xbar transpose (bf16); nsq per node via fused
        scalar_tensor_tensor accum; rinv = exp(-0.5*ln(nsq+eps)) (no table
        switch); z^T = si^T * bcast(rinv) on GPSIMD (bcast via 0-stride DMA
        through a DRAM bounce).
      * Block-diagonal gram: fragments are block-aligned, so same-fragment
        pairs live in 9 diagonal [128 x 128] tiles (not an N^2 gram):
        G tile = z^T[:,blk].T @ z^T[:,blk] on PE.
      * (G-C)^2 via ACT Square straight off PSUM; masked pair-error row sums
        in one fused scalar_tensor_tensor (x*mask, accum add) per tile; the
        host mask replicates same-fragment & j>i & real-node semantics
        exactly.
      * Per-fragment sums of si via one-hot matmuls into PSUM.
"""

import numpy as np
import ml_dtypes

N = 8192
H = 256
NF = 256
LOG2 = 0.6931471805599453
R = 0.3
C = 0.8
VW = 0.5

NCORES = 8
NFL = NF // NCORES          # 32 fragments per device
NCAP = 1152                 # padded nodes per device (9 blocks suffice
                            # for the bin-packed fragment layout)
NCH = NCAP // 128           # 9 chunks of 128 nodes

BF = ml_dtypes.bfloat16

_PROG_CACHE = {}


def _patch_act_tables():
    """Force the ACT table-load inserter to use the one set containing
    Exp+Ln+Square (it otherwise thrashes between exp_and_others and
    natural_log, ~2.7us per reload). Set ids are positional, so keep all
    keys and just empty the other sets."""
    import functools
    import concourse.hw_specs as hw_specs
    import concourse.bacc as bacc_mod

    if getattr(hw_specs.get_activation_tables, "_ecc_patched", False):
        return
    orig = hw_specs.get_activation_tables
    KEEP = "natural_log_exp_and_others"

    @functools.cache
    def patched(module_arch):
        t = orig(module_arch)
        return {k: (v if k == KEEP else set()) for k, v in t.items()}

    patched._ecc_patched = True
    hw_specs.get_activation_tables = patched
    if hasattr(bacc_mod, "get_activation_tables"):
        bacc_mod.get_activation_tables = patched


def _build_program():
    from contextlib import ExitStack
    import concourse.bacc as bacc
    import concourse.bass as bass_mod
    import concourse.tile as tile
    import concourse.mybir as mybir

    _patch_act_tables()

    dt = mybir.dt
    f32, bf = dt.float32, dt.bfloat16
    AF = mybir.ActivationFunctionType
    ALU = mybir.AluOpType

    nc = bacc.Bacc("TRN2", target_bir_lowering=False, debug=False)

    # ---- DRAM I/O ----
    xin = nc.dram_tensor("xin", [4, 4, 128, NCAP], bf, kind="ExternalInput")
    w1 = nc.dram_tensor("w1", [2, 4, 128, 256], bf, kind="ExternalInput")
    w2 = nc.dram_tensor("w2", [2, 2, 128, 256], bf, kind="ExternalInput")
    b2 = nc.dram_tensor("b2", [2, 2, 128], f32, kind="ExternalInput")
    b1 = nc.dram_tensor("b1", [2, 128, 1], f32, kind="ExternalInput")
    mskt = nc.dram_tensor("mskt", [128, NCH * 128], bf, kind="ExternalInput")
    oneh = nc.dram_tensor("oneh", [NCH, 128, NFL], bf, kind="ExternalInput")

    seg = nc.dram_tensor("seg", [NFL, 4 * 256], f32, kind="ExternalOutput")
    errc = nc.dram_tensor("errc", [4, 128, NCH], f32, kind="ExternalOutput")
    nsqo = nc.dram_tensor("nsqo", [4, 128, NCH], f32, kind="ExternalOutput")

    NCHUNKS = [(0, 512), (512, 512), (1024, NCAP - 1024)]

    with tile.TileContext(nc) as tc:
        with ExitStack() as ctx:
            const = ctx.enter_context(tc.tile_pool(name="const", bufs=1))
            hs_p = ctx.enter_context(tc.tile_pool(name="hs", bufs=6))
            sit_p = ctx.enter_context(tc.tile_pool(name="sit", bufs=8))
            zt_p = ctx.enter_context(tc.tile_pool(name="zt", bufs=8))
            nm_p = ctx.enter_context(tc.tile_pool(name="nm", bufs=4))
            f_p = ctx.enter_context(tc.tile_pool(name="fp", bufs=4))
            rbc_p = ctx.enter_context(tc.tile_pool(name="rbc", bufs=3))
            st_p = ctx.enter_context(tc.tile_pool(name="st", bufs=6))
            ps = ctx.enter_context(tc.tile_pool(name="ps", bufs=3, space="PSUM"))
            psg = ctx.enter_context(tc.tile_pool(name="psg", bufs=2, space="PSUM"))
            dr = ctx.enter_context(tc.tile_pool(name="dr", bufs=2, space="DRAM"))

            # ---- constants to SBUF (split across SP-HWDGE and Pool-SWDGE);
            # L1(m0) dependencies (w1[0], xin[0,k]) issued first per queue ----
            w1_sb = const.tile([128, 2, 4, 256], bf, tag="w1")
            w2_sb = const.tile([128, 2, 2, 256], bf, tag="w2")
            xin_sb = const.tile([128, 4, 4, NCAP], bf, tag="xin")
            nc.gpsimd.dma_start(
                out=w1_sb[:, 0], in_=w1[0].rearrange("k p f -> p k f")
            )
            for k in range(4):
                nc.sync.dma_start(out=xin_sb[:, 0, k], in_=xin[0, k])
            nc.gpsimd.dma_start(
                out=w1_sb[:, 1], in_=w1[1].rearrange("k p f -> p k f")
            )
            for b in range(2):
                nc.sync.dma_start(
                    out=w2_sb[:, b], in_=w2[b].rearrange("k p f -> p k f")
                )
            b2_sb = const.tile([128, 2, 2], f32, tag="b2")
            nc.sync.dma_start(out=b2_sb, in_=b2.rearrange("b o p -> p b o"))
            b1_sb = const.tile([128, 2], f32, tag="b1")
            for mc in range(2):
                nc.sync.dma_start(out=b1_sb[:, mc : mc + 1], in_=b1[mc])
            for m in range(1, 4):
                eng = nc.sync if m % 2 == 0 else nc.gpsimd
                for k in range(4):
                    eng.dma_start(out=xin_sb[:, m, k], in_=xin[m, k])
            msk_sb = const.tile([128, NCH * 128], bf, tag="msk")
            nc.gpsimd.dma_start(out=msk_sb[:, :], in_=mskt[:, :])
            oneh_sb = const.tile([128, NCH, NFL], bf, tag="oneh")
            nc.gpsimd.dma_start(
                out=oneh_sb, in_=oneh.rearrange("c p f -> p c f")
            )
            eps_sb = const.tile([128, 1], f32, tag="epsc")
            nc.gpsimd.memset(eps_sb, 1e-24)
            negc_sb = const.tile([128, 1], f32, tag="negc")
            nc.gpsimd.memset(negc_sb, -C)

            si_nm = []  # node-major si per matrix, resident for segsum
            sgs = const.tile([NFL, 4, 256], f32, tag="sgs")

            zts = []
            sits = []

            # ======== FRONT: MLP for all matrices ========
            for m in range(4):
                wsel = 0 if m == 0 else 1
                # ---- L1: h^T strips per m-chunk, then Softplus ----
                hs_t = []
                for mc in range(2):
                    # strip split [0:1024]+[1024:1152]: 1024*f32 = exactly 2
                    # PSUM banks, so 3 strip slots fit (vs 2 padded 3-bank)
                    ph = ps.tile([128, 1024], f32, tag="strip")
                    pht = psg.tile([128, NCAP - 1024], f32, tag="gram")
                    # k outer: one LDWEIGHTS per K-chunk, streamed over all N
                    for k in range(4):
                        for n0, nw in NCHUNKS:
                            dst = (
                                ph[:, n0 : n0 + nw]
                                if n0 < 1024
                                else pht[:, :nw]
                            )
                            nc.tensor.matmul(
                                dst,
                                w1_sb[:, wsel, k, 128 * mc : 128 * (mc + 1)],
                                xin_sb[:, m, k, n0 : n0 + nw],
                                start=(k == 0),
                                stop=(k == 3),
                            )
                    hst = hs_p.tile([128, NCAP], bf, tag="hs")
                    et = hs_p.tile([128, NCAP], f32, tag="et")
                    bias = b1_sb[:, mc : mc + 1] if m == 0 else 0.0
                    # softplus(x+b1) = ln(exp(x+b1) + 1)
                    nc.scalar.activation(et[:, :1024], ph, AF.Exp, bias=bias, scale=1.0)
                    nc.scalar.activation(et[:, 1024:], pht, AF.Exp, bias=bias, scale=1.0)
                    nc.scalar.activation(hst, et, AF.Ln, bias=1.0, scale=1.0)
                    hs_t.append(hst)

                # ---- L2: si^T strips (+ bias row via K=1 matmul) ----
                sit_t = []
                for mc in range(2):
                    psi = ps.tile([128, 1024], f32, tag="strip")
                    psit = psg.tile([128, NCAP - 1024], f32, tag="gram")
                    for k in range(2):
                        for n0, nw in NCHUNKS:
                            dst = (
                                psi[:, n0 : n0 + nw]
                                if n0 < 1024
                                else psit[:, :nw]
                            )
                            nc.tensor.matmul(
                                dst,
                                w2_sb[:, wsel, k, 128 * mc : 128 * (mc + 1)],
                                hs_t[k][:, n0 : n0 + nw],
                                start=(k == 0),
                                stop=(k == 1),
                            )
                    sit = sit_p.tile([128, NCAP], bf, tag="sit")
                    # drain + b2' bias add (per-partition scalar in feat-major)
                    nc.vector.tensor_scalar(
                        sit[:, :1024], psi, b2_sb[:, wsel, mc : mc + 1], None, ALU.add
                    )
                    nc.vector.tensor_scalar(
                        sit[:, 1024:], psit, b2_sb[:, wsel, mc : mc + 1], None, ALU.add
                    )
                    sit_t.append(sit)

                # ---- node-major si via DMA xbar transpose ----
                snm = nm_p.tile([128, NCH, 256], bf, tag="sinm")
                for mc in range(2):
                    nc.sync.dma_start_transpose(
                        snm[:, :, 128 * mc : 128 * (mc + 1)], sit_t[mc][:, :]
                    )
                si_nm.append(snm)
                sits.append(sit_t)

            # ======== BACK: normalize + gram + masked reduce + seg sums ====
            for m in range(4):
                snm = si_nm[m]
                sit_t = sits[m]
                # ---- nsq via fused multiply+accumulate ----
                nsq = st_p.tile([128, NCH], f32, tag="nsq")
                for c in range(NCH):
                    trash = st_p.tile([128, 256], bf, tag="trash")
                    nc.vector.scalar_tensor_tensor(
                        out=trash,
                        in0=snm[:, c, :],
                        scalar=1.0,
                        in1=snm[:, c, :],
                        op0=ALU.mult,
                        op1=ALU.mult,
                        accum_out=nsq[:, c : c + 1],
                    )
                nc.gpsimd.dma_start(out=nsqo[m], in_=nsq)
                # rinv = (nsq+eps)^-0.5 = exp(-0.5*ln(nsq+eps)); single ACT set
                lnq = st_p.tile([128, NCH], f32, tag="nrm")
                nc.scalar.activation(lnq, nsq, AF.Ln, bias=eps_sb[:, 0:1], scale=1.0)
                rinvb = st_p.tile([128, NCH], bf, tag="rinvb")
                nc.scalar.activation(rinvb, lnq, AF.Exp, bias=0.0, scale=-0.5)

                # ---- rbc[p, 128c+pi] = rinv[pi, c] via DRAM-bounce bcast DMA ----
                rdr = dr.tile([NCH, 128], bf, tag="rdr")
                nc.sync.dma_start(
                    out=rdr.rearrange("c p -> p c"), in_=rinvb[:, :]
                )
                rbc = rbc_p.tile([128, NCH, 128], bf, tag="rbc")
                rsrc = bass_mod.AP(
                    tensor=rdr.tensor,
                    offset=rdr.offset,
                    ap=[[0, 128], [1, NCH * 128]],
                )
                nc.sync.dma_start(out=rbc[:, :, :], in_=rsrc)

                # ---- z^T = si^T * rbc ----
                zt_t = []
                for mc in range(2):
                    zt = zt_p.tile([128, NCAP], bf, tag="zt")
                    eng = nc.gpsimd
                    eng.tensor_tensor(
                        out=zt,
                        in0=sit_t[mc],
                        in1=rbc.rearrange("p c q -> p (c q)"),
                        op=ALU.mult,
                    )
                    zt_t.append(zt)
                zts.append(zt_t)

            # ---- banded gram + masked reduce + seg sums ----
            errts = []
            for _mi in range(4):
                errt_tile = st_p.tile([128, NCH], f32, tag="errt")
                errts.append(errt_tile)
            # fragments are block-aligned (host bin-packing), so each pair
            # tile is a single [128,128] diagonal block
            GRP = [(0, 4), (4, 4), (8, 1)]  # strip = (first tile, #tiles)
            for m in range(4):
                for g0, gn in GRP:
                    zt_t = zts[m]
                    gps = psg.tile([128, 512], f32, tag="gram")
                    for bi in range(gn):
                        t = g0 + bi
                        for k in range(2):
                            nc.tensor.matmul(
                                gps[:, 128 * bi : 128 * (bi + 1)],
                                zt_t[k][:, 128 * t : 128 * (t + 1)],
                                zt_t[k][:, 128 * t : 128 * (t + 1)],
                                start=(k == 0),
                                stop=(k == 1),
                            )
                    fsq = f_p.tile([128, 512], bf, tag="ft")
                    nc.scalar.activation(
                        fsq[:, : 128 * gn],
                        gps[:, : 128 * gn],
                        AF.Square,
                        bias=negc_sb[:, 0:1],
                        scale=1.0,
                    )
                    for bi in range(gn):
                        t = g0 + bi
                        trash2 = st_p.tile([128, 128], bf, tag="trash")
                        nc.vector.scalar_tensor_tensor(
                            out=trash2,
                            in0=fsq[:, 128 * bi : 128 * (bi + 1)],
                            scalar=1.0,
                            in1=msk_sb[:, 128 * t : 128 * (t + 1)],
                            op0=ALU.mult,
                            op1=ALU.mult,
                            accum_out=errts[m][:, t : t + 1],
                        )
            for m in range(4):
                nc.gpsimd.dma_start(out=errc[m], in_=errts[m])
                # ---- segment sums via one-hot matmuls (MLP slots idle now)
                sgp = ps.tile([NFL, 256], f32, tag="strip")
                for c in range(NCH):
                    nc.tensor.matmul(
                        sgp,
                        oneh_sb[:, c, :],
                        si_nm[m][:, c, :],
                        start=(c == 0),
                        stop=(c == NCH - 1),
                    )
                nc.vector.tensor_copy(sgs[:, m], sgp)

            nc.sync.dma_start(
                out=seg[:, :], in_=sgs.rearrange("f m h -> f (m h)")
            )

    nc.compile()
    return nc


def _host_prep(inputs):
    """Sort/shard/pad on host; build per-core input dicts."""
    frag = np.asarray(inputs["fragment_ids"]).astype(np.int64)
    counts = np.bincount(frag, minlength=NF).astype(np.int64)

    W1 = np.asarray(inputs["W1"], np.float32)
    W2 = np.asarray(inputs["W2"], np.float32)
    V1 = np.asarray(inputs["V1"], np.float32)
    V2 = np.asarray(inputs["V2"], np.float32)
    b1 = np.asarray(inputs["b1"], np.float32)
    b2 = np.asarray(inputs["b2"], np.float32)

    # weights, lerp folded into stacked W1'; ShiftedSoftplus -log2 into b2'
    w1s = np.concatenate([R * W1, (1.0 - R) * W1], axis=0)      # [512, 256]
    w1v = np.concatenate([R * V1, (1.0 - R) * V1], axis=0)
    b2s = (b2 - LOG2 * W2.sum(axis=0)).astype(np.float32)
    b2v = (-LOG2 * V2.sum(axis=0)).astype(np.float32)

    w1_arr = np.stack([w1s, w1v]).reshape(2, 4, 128, 256).astype(BF)
    w2_arr = np.stack([W2, V2]).reshape(2, 2, 128, 256).astype(BF)
    b2_arr = np.stack([b2s, b2v]).reshape(2, 2, 128).astype(np.float32)
    b1_arr = b1.reshape(2, 128, 1).astype(np.float32)  # [mc, 128, 1]

    ss = np.asarray(inputs["scalar_short"], np.float32)
    sl = np.asarray(inputs["scalar_long"], np.float32)
    vs = np.asarray(inputs["vector_short"], np.float32)
    vl = np.asarray(inputs["vector_long"], np.float32)

    assert counts.max() <= 128, "fragment larger than one block"
    order = np.argsort(frag, kind="stable")
    fstart = np.zeros(NF + 1, np.int64)
    np.cumsum(counts, out=fstart[1:])

    in_maps = []
    meta = []
    for d in range(NCORES):
        g0, g1 = d * NFL, (d + 1) * NFL
        # Bin-pack this device's fragments into 128-node blocks (first-fit
        # decreasing) so no fragment crosses a block boundary: pair tiles
        # then reduce to single [128,128] diagonal blocks.
        fcnt = [(int(counts[g0 + f]), f) for f in range(NFL)]
        fcnt.sort(reverse=True)
        blocks = []  # list of (used, [frag...])
        for c, f in fcnt:
            if c == 0:
                continue
            for blk in blocks:
                if blk[0] + c <= 128:
                    blk[0] += c
                    blk[1].append(f)
                    break
            else:
                blocks.append([c, [f]])
        assert len(blocks) <= NCH, f"device {d}: {len(blocks)} blocks > {NCH}"
        # node order: block by block, fragments whole, pad each block to 128
        nodes = np.zeros(NCAP, np.int64)
        floc = np.full(NCAP, -1, np.int64)
        spans = [None] * NFL  # local frag -> (start, count)
        real = np.zeros(NCAP, bool)
        for bi, (_, fl) in enumerate(blocks):
            pos = bi * 128
            for f in fl:
                g = g0 + f
                c = int(counts[g])
                nodes[pos : pos + c] = order[fstart[g] : fstart[g] + c]
                floc[pos : pos + c] = f
                real[pos : pos + c] = True
                spans[f] = (pos, c)
                pos += c
        nd = NCAP

        def padT(x):  # gather+transpose -> [256, NCAP] bf16, pads zero
            out = np.zeros((256, NCAP), np.float32)
            out[:, real] = x[nodes[real]].T
            return out

        xs = [
            np.concatenate([padT(ss), padT(sl)], axis=0),
            np.concatenate([padT(vs[:, 0, :]), padT(vl[:, 0, :])], axis=0),
            np.concatenate([padT(vs[:, 1, :]), padT(vl[:, 1, :])], axis=0),
            np.concatenate([padT(vs[:, 2, :]), padT(vl[:, 2, :])], axis=0),
        ]
        xin = np.stack(xs).reshape(4, 4, 128, NCAP).astype(BF)

        # block-diagonal mask: tile t = [i in block t, j in block t]
        msk = np.zeros((128, NCH, 128), np.float32)
        for t in range(NCH):
            idxs = 128 * t + np.arange(128)
            fi = floc[idxs]
            msk[:, t, :] = (
                (fi[:, None] >= 0)
                & (fi[None, :] >= 0)
                & (fi[:, None] == fi[None, :])
                & (idxs[None, :] > idxs[:, None])
            )
        msk_arr = msk.reshape(128, NCH * 128).astype(BF)

        oh = np.zeros((NCH, 128, NFL), np.float32)
        idx = np.arange(NCAP)[real]
        oh[idx // 128, idx % 128, floc[idx]] = 1.0
        oh_arr = oh.astype(BF)

        in_maps.append(
            {
                "xin": xin,
                "w1": w1_arr,
                "w2": w2_arr,
                "b2": b2_arr,
                "b1": b1_arr,
                "mskt": msk_arr,
                "oneh": oh_arr,
            }
        )
        meta.append((g0, spans))
    return in_maps, meta, counts


def _host_combine(results, meta, counts):
    total = 0.0
    nvalid = 0
    for d in range(NCORES):
        seg = np.asarray(results[d]["seg"], np.float64)      # [NFL, 1024]
        errc = np.asarray(results[d]["errc"], np.float64)    # [4, 128, NCH]
        nsqo = np.asarray(results[d]["nsqo"], np.float64)    # [4, 128, NCH]
        err_nodes = errc.transpose(0, 2, 1).reshape(4, NCAP)
        nsq_nodes = nsqo.transpose(0, 2, 1).reshape(4, NCAP)
        g0, spans = meta[d]
        # per-local-fragment sums over nodes
        for f in range(NFL):
            c = int(counts[g0 + f])
            if c == 0:
                continue
            pos, c2 = spans[f]
            assert c2 == c
            sl_ = slice(pos, pos + c)
            pc = c * (c - 1) * 0.5
            pcs = max(pc, 1.0)
            ns = float(c)

            xsq_s = nsq_nodes[0, sl_].sum()
            Ssq = (seg[f, 0:256] ** 2).sum()
            s_var = (xsq_s - Ssq / ns) / ns
            s_sim = err_nodes[0, sl_].sum() / pcs
            scalar_loss = s_var + s_sim

            xsq_v = nsq_nodes[1:4, sl_].sum()
            Vsq = (seg[f, 256:1024] ** 2).sum()
            v_var = (xsq_v - Vsq / ns) / ns
            dir_loss = err_nodes[1:4, sl_].sum() / (3.0 * pcs)
            vector_loss = v_var + dir_loss

            if pc > 0:
                total += (1.0 - VW) * scalar_loss + VW * vector_loss
                nvalid += 1
    if nvalid == 0:
        return np.float32(0.0)
    return np.float32(total / nvalid)


TRACE = False
LAST_RESULT = None


def kernel(**inputs):
    global LAST_RESULT
    if "nc" not in _PROG_CACHE:
        _PROG_CACHE["nc"] = _build_program()
    nc = _PROG_CACHE["nc"]

    in_maps, meta, counts = _host_prep(inputs)

    from concourse.bass_utils import run_bass_kernel_spmd

    res = run_bass_kernel_spmd(
        nc, in_maps, core_ids=list(range(NCORES)), trace=TRACE
    )
    LAST_RESULT = res
    return _host_combine(res.results, meta, counts)


if __name__ == "__main__":
    import reference

    inputs = {k: np.asarray(v) for k, v in reference.setup_inputs().items()}
    out = kernel(**inputs)
    print("kernel out:", out)


# revision 81
# speedup vs baseline: 1.0048x; 1.0048x over previous
"""Trainium2 Bass kernel for nn_EnhancedConformationalConsistencyLoss.

Strategy (segment_reduce, 8 cores):
  - Host: stable-sort nodes by fragment id; fragments f in [32*d, 32*(d+1))
    go to device d, then bin-packed (first-fit decreasing) into 128-node
    partition blocks so no fragment crosses a block boundary; 9 blocks
    (NCAP=1152) suffice. O(N) index prep + O(NF) final combine only; all
    O(N*H^2)/O(pairs) work is on-device.
  - Device (per core, SPMD with per-core shards), two phases so PE streams
    continuously ("front" = MLP for all 4 feature matrices, "back" =
    normalize + pair terms):
      * MLP feature-major, weights stationary: h^T = W1'.T @ [short^T;long^T]
        with the short/long lerp folded into the K=512-stacked scaled W1';
        softplus as Ln(Exp(x+b1)+1) (both + Square live in one ACT table set,
        table thrash patched out); si^T = W2.T @ sp, b2' (carrying the
        ShiftedSoftplus -log2) added during the PSUM drain as a per-partition
        tensor_scalar.
      * si node-major via DMA xbar transpose (bf16); nsq per node via fused
        scalar_tensor_tensor accum; rinv = exp(-0.5*ln(nsq+eps)) (no table
        switch); z^T = si^T * bcast(rinv) on GPSIMD (bcast via 0-stride DMA
        through a DRAM bounce).
      * Block-diagonal gram: fragments are block-aligned, so same-fragment
        pairs live in 9 diagonal [128 x 128] tiles (not an N^2 gram):
        G tile = z^T[:,blk].T @ z^T[:,blk] on PE.
      * (G-C)^2 via ACT Square straight off PSUM; masked pair-error row sums
        in one fused scalar_tensor_tensor (x*mask, accum add) per tile; the
        host mask replicates same-fragment & j>i & real-node semantics
        exactly.
      * Per-fragment sums of si via one-hot matmuls into PSUM.
"""

import numpy as np
import ml_dtypes

N = 8192
H = 256
NF = 256
LOG2 = 0.6931471805599453
R = 0.3
C = 0.8
VW = 0.5

NCORES = 8
NFL = NF // NCORES          # 32 fragments per device
NCAP = 1152                 # padded nodes per device (9 blocks suffice
                            # for the bin-packed fragment layout)
NCH = NCAP // 128           # 9 chunks of 128 nodes

BF = ml_dtypes.bfloat16

_PROG_CACHE = {}


def _patch_act_tables():
    """Force the ACT table-load inserter to use the one set containing
    Exp+Ln+Square (it otherwise thrashes between exp_and_others and
    natural_log, ~2.7us per reload). Set ids are positional, so keep all
    keys and just empty the other sets."""
    import functools
    import concourse.hw_specs as hw_specs
    import concourse.bacc as bacc_mod

    if getattr(hw_specs.get_activation_tables, "_ecc_patched", False):
        return
    orig = hw_specs.get_activation_tables
    KEEP = "natural_log_exp_and_others"

    @functools.cache
    def patched(module_arch):
        t = orig(module_arch)
        return {k: (v if k == KEEP else set()) for k, v in t.items()}

    patched._ecc_patched = True
    hw_specs.get_activation_tables = patched
    if hasattr(bacc_mod, "get_activation_tables"):
        bacc_mod.get_activation_tables = patched


def _build_program():
    from contextlib import ExitStack
    import concourse.bacc as bacc
    import concourse.bass as bass_mod
    import concourse.tile as tile
    import concourse.mybir as mybir

    _patch_act_tables()

    dt = mybir.dt
    f32, bf = dt.float32, dt.bfloat16
    AF = mybir.ActivationFunctionType
    ALU = mybir.AluOpType

    nc = bacc.Bacc("TRN2", target_bir_lowering=False, debug=False)

    # ---- DRAM I/O ----
    xin = nc.dram_tensor("xin", [4, 4, 128, NCAP], bf, kind="ExternalInput")
    w1 = nc.dram_tensor("w1", [2, 4, 128, 256], bf, kind="ExternalInput")
    w2 = nc.dram_tensor("w2", [2, 2, 128, 256], bf, kind="ExternalInput")
    b2 = nc.dram_tensor("b2", [2, 2, 128], f32, kind="ExternalInput")
    b1 = nc.dram_tensor("b1", [2, 128, 1], f32, kind="ExternalInput")
    mskt = nc.dram_tensor("mskt", [128, NCH * 128], bf, kind="ExternalInput")
    oneh = nc.dram_tensor("oneh", [NCH, 128, NFL], bf, kind="ExternalInput")

    seg = nc.dram_tensor("seg", [NFL, 4 * 256], f32, kind="ExternalOutput")
    errc = nc.dram_tensor("errc", [4, 128, NCH], f32, kind="ExternalOutput")
    nsqo = nc.dram_tensor("nsqo", [4, 128, NCH], f32, kind="ExternalOutput")

    NCHUNKS = [(0, 512), (512, 512), (1024, NCAP - 1024)]

    with tile.TileContext(nc) as tc:
        with ExitStack() as ctx:
            const = ctx.enter_context(tc.tile_pool(name="const", bufs=1))
            hs_p = ctx.enter_context(tc.tile_pool(name="hs", bufs=6))
            sit_p = ctx.enter_context(tc.tile_pool(name="sit", bufs=8))
            zt_p = ctx.enter_context(tc.tile_pool(name="zt", bufs=8))
            nm_p = ctx.enter_context(tc.tile_pool(name="nm", bufs=4))
            f_p = ctx.enter_context(tc.tile_pool(name="fp", bufs=4))
            rbc_p = ctx.enter_context(tc.tile_pool(name="rbc", bufs=3))
            st_p = ctx.enter_context(tc.tile_pool(name="st", bufs=6))
            ps = ctx.enter_context(tc.tile_pool(name="ps", bufs=3, space="PSUM"))
            psg = ctx.enter_context(tc.tile_pool(name="psg", bufs=2, space="PSUM"))
            dr = ctx.enter_context(tc.tile_pool(name="dr", bufs=2, space="DRAM"))

            # ---- constants to SBUF (split across SP-HWDGE and Pool-SWDGE);
            # L1(m0) dependencies (w1[0], xin[0,k]) issued first per queue ----
            w1_sb = const.tile([128, 2, 4, 256], bf, tag="w1")
            w2_sb = const.tile([128, 2, 2, 256], bf, tag="w2")
            xin_sb = const.tile([128, 4, 4, NCAP], bf, tag="xin")
            nc.gpsimd.dma_start(
                out=w1_sb[:, 0], in_=w1[0].rearrange("k p f -> p k f")
            )
            for k in range(4):
                nc.sync.dma_start(out=xin_sb[:, 0, k], in_=xin[0, k])
            nc.gpsimd.dma_start(
                out=w1_sb[:, 1], in_=w1[1].rearrange("k p f -> p k f")
            )
            for b in range(2):
                nc.sync.dma_start(
                    out=w2_sb[:, b], in_=w2[b].rearrange("k p f -> p k f")
                )
            b2_sb = const.tile([128, 2, 2], f32, tag="b2")
            nc.sync.dma_start(out=b2_sb, in_=b2.rearrange("b o p -> p b o"))
            b1_sb = const.tile([128, 2], f32, tag="b1")
            for mc in range(2):
                nc.sync.dma_start(out=b1_sb[:, mc : mc + 1], in_=b1[mc])
            for m in range(1, 4):
                eng = nc.sync if m % 2 == 0 else nc.gpsimd
                for k in range(4):
                    eng.dma_start(out=xin_sb[:, m, k], in_=xin[m, k])
            msk_sb = const.tile([128, NCH * 128], bf, tag="msk")
            nc.gpsimd.dma_start(out=msk_sb[:, :], in_=mskt[:, :])
            oneh_sb = const.tile([128, NCH, NFL], bf, tag="oneh")
            nc.gpsimd.dma_start(
                out=oneh_sb, in_=oneh.rearrange("c p f -> p c f")
            )
            eps_sb = const.tile([128, 1], f32, tag="epsc")
            nc.gpsimd.memset(eps_sb, 1e-24)
            negc_sb = const.tile([128, 1], f32, tag="negc")
            nc.gpsimd.memset(negc_sb, -C)

            si_nm = []  # node-major si per matrix, resident for segsum
            sgs = const.tile([NFL, 4, 256], f32, tag="sgs")

            zts = []
            sits = []

            # ======== FRONT: MLP for all matrices ========
            for m in range(4):
                wsel = 0 if m == 0 else 1
                # ---- L1: h^T strips per m-chunk, then Softplus ----
                hs_t = []
                for mc in range(2):
                    # strip split [0:1024]+[1024:1152]: 1024*f32 = exactly 2
                    # PSUM banks, so 3 strip slots fit (vs 2 padded 3-bank)
                    ph = ps.tile([128, 1024], f32, tag="strip")
                    pht = psg.tile([128, NCAP - 1024], f32, tag="gram")
                    # k outer: one LDWEIGHTS per K-chunk, streamed over all N
                    for k in range(4):
                        for n0, nw in NCHUNKS:
                            dst = (
                                ph[:, n0 : n0 + nw]
                                if n0 < 1024
                                else pht[:, :nw]
                            )
                            nc.tensor.matmul(
                                dst,
                                w1_sb[:, wsel, k, 128 * mc : 128 * (mc + 1)],
                                xin_sb[:, m, k, n0 : n0 + nw],
                                start=(k == 0),
                                stop=(k == 3),
                            )
                    hst = hs_p.tile([128, NCAP], bf, tag="hs")
                    et = hs_p.tile([128, NCAP], f32, tag="et")
                    bias = b1_sb[:, mc : mc + 1] if m == 0 else 0.0
                    # softplus(x+b1) = ln(exp(x+b1) + 1)
                    nc.scalar.activation(et[:, :1024], ph, AF.Exp, bias=bias, scale=1.0)
                    nc.scalar.activation(et[:, 1024:], pht, AF.Exp, bias=bias, scale=1.0)
                    nc.scalar.activation(hst, et, AF.Ln, bias=1.0, scale=1.0)
                    hs_t.append(hst)

                # ---- L2: si^T strips (+ bias row via K=1 matmul) ----
                sit_t = []
                for mc in range(2):
                    psi = ps.tile([128, 1024], f32, tag="strip")
                    psit = psg.tile([128, NCAP - 1024], f32, tag="gram")
                    for k in range(2):
                        for n0, nw in NCHUNKS:
                            dst = (
                                psi[:, n0 : n0 + nw]
                                if n0 < 1024
                                else psit[:, :nw]
                            )
                            nc.tensor.matmul(
                                dst,
                                w2_sb[:, wsel, k, 128 * mc : 128 * (mc + 1)],
                                hs_t[k][:, n0 : n0 + nw],
                                start=(k == 0),
                                stop=(k == 1),
                            )
                    sit = sit_p.tile([128, NCAP], bf, tag="sit")
                    # drain + b2' bias add (per-partition scalar in feat-major)
                    nc.vector.tensor_scalar(
                        sit[:, :1024], psi, b2_sb[:, wsel, mc : mc + 1], None, ALU.add
                    )
                    nc.vector.tensor_scalar(
                        sit[:, 1024:], psit, b2_sb[:, wsel, mc : mc + 1], None, ALU.add
                    )
                    sit_t.append(sit)

                # ---- node-major si via DMA xbar transpose ----
                snm = nm_p.tile([128, NCH, 256], bf, tag="sinm")
                for mc in range(2):
                    nc.sync.dma_start_transpose(
                        snm[:, :, 128 * mc : 128 * (mc + 1)], sit_t[mc][:, :]
                    )
                si_nm.append(snm)
                sits.append(sit_t)

            # ======== BACK: normalize + gram + masked reduce + seg sums ====
            for m in range(4):
                snm = si_nm[m]
                sit_t = sits[m]
                # ---- nsq via fused multiply+accumulate ----
                nsq = st_p.tile([128, NCH], f32, tag="nsq")
                for c in range(NCH):
                    trash = st_p.tile([128, 256], bf, tag="trash")
                    nc.vector.scalar_tensor_tensor(
                        out=trash,
                        in0=snm[:, c, :],
                        scalar=1.0,
                        in1=snm[:, c, :],
                        op0=ALU.mult,
                        op1=ALU.mult,
                        accum_out=nsq[:, c : c + 1],
                    )
                nc.gpsimd.dma_start(out=nsqo[m], in_=nsq)
                # rinv = (nsq+eps)^-0.5 = exp(-0.5*ln(nsq+eps)); single ACT set
                lnq = st_p.tile([128, NCH], f32, tag="nrm")
                nc.scalar.activation(lnq, nsq, AF.Ln, bias=eps_sb[:, 0:1], scale=1.0)
                rinvb = st_p.tile([128, NCH], bf, tag="rinvb")
                nc.scalar.activation(rinvb, lnq, AF.Exp, bias=0.0, scale=-0.5)

                # ---- rbc[p, 128c+pi] = rinv[pi, c] via DRAM-bounce bcast DMA ----
                rdr = dr.tile([NCH, 128], bf, tag="rdr")
                nc.sync.dma_start(
                    out=rdr.rearrange("c p -> p c"), in_=rinvb[:, :]
                )
                rbc = rbc_p.tile([128, NCH, 128], bf, tag="rbc")
                rsrc = bass_mod.AP(
                    tensor=rdr.tensor,
                    offset=rdr.offset,
                    ap=[[0, 128], [1, NCH * 128]],
                )
                nc.sync.dma_start(out=rbc[:, :, :], in_=rsrc)

                # ---- z^T = si^T * rbc ----
                zt_t = []
                for mc in range(2):
                    zt = zt_p.tile([128, NCAP], bf, tag="zt")
                    eng = nc.gpsimd
                    eng.tensor_tensor(
                        out=zt,
                        in0=sit_t[mc],
                        in1=rbc.rearrange("p c q -> p (c q)"),
                        op=ALU.mult,
                    )
                    zt_t.append(zt)
                zts.append(zt_t)

            # ---- banded gram + masked reduce + seg sums ----
            errts = []
            for _mi in range(4):
                errt_tile = st_p.tile([128, NCH], f32, tag="errt")
                errts.append(errt_tile)
            # fragments are block-aligned (host bin-packing), so each pair
            # tile is a single [128,128] diagonal block
            GRP = [(0, 4), (4, 4), (8, 1)]  # strip = (first tile, #tiles)
            for m in range(4):
                for g0, gn in GRP:
                    zt_t = zts[m]
                    gps = psg.tile([128, 512], f32, tag="gram")
                    for bi in range(gn):
                        t = g0 + bi
                        for k in range(2):
                            nc.tensor.matmul(
                                gps[:, 128 * bi : 128 * (bi + 1)],
                                zt_t[k][:, 128 * t : 128 * (t + 1)],
                                zt_t[k][:, 128 * t : 128 * (t + 1)],
                                start=(k == 0),
                                stop=(k == 1),
                            )
                    fsq = f_p.tile([128, 512], bf, tag="ft")
                    nc.scalar.activation(
                        fsq[:, : 128 * gn],
                        gps[:, : 128 * gn],
                        AF.Square,
                        bias=negc_sb[:, 0:1],
                        scale=1.0,
                    )
                    for bi in range(gn):
                        t = g0 + bi
                        trash2 = st_p.tile([128, 128], bf, tag="trash")
                        nc.vector.scalar_tensor_tensor(
                            out=trash2,
                            in0=fsq[:, 128 * bi : 128 * (bi + 1)],
                            scalar=1.0,
                            in1=msk_sb[:, 128 * t : 128 * (t + 1)],
                            op0=ALU.mult,
                            op1=ALU.mult,
                            accum_out=errts[m][:, t : t + 1],
                        )
            for m in range(4):
                nc.gpsimd.dma_start(out=errc[m], in_=errts[m])
                # ---- segment sums via one-hot matmuls (MLP slots idle now)
                sgp = ps.tile([NFL, 256], f32, tag="strip")
                for c in range(NCH):
                    nc.tensor.matmul(
                        sgp,
                        oneh_sb[:, c, :],
                        si_nm[m][:, c, :],
                        start=(c == 0),
                        stop=(c == NCH - 1),
                    )
                nc.scalar.copy(sgs[:, m], sgp)

            nc.sync.dma_start(
                out=seg[:, :], in_=sgs.rearrange("f m h -> f (m h)")
            )

    nc.compile()
    return nc


def _host_prep(inputs):
    """Sort/shard/pad on host; build per-core input dicts."""
    frag = np.asarray(inputs["fragment_ids"]).astype(np.int64)
    counts = np.bincount(frag, minlength=NF).astype(np.int64)

    W1 = np.asarray(inputs["W1"], np.float32)
    W2 = np.asarray(inputs["W2"], np.float32)
    V1 = np.asarray(inputs["V1"], np.float32)
    V2 = np.asarray(inputs["V2"], np.float32)
    b1 = np.asarray(inputs["b1"], np.float32)
    b2 = np.asarray(inputs["b2"], np.float32)

    # weights, lerp folded into stacked W1'; ShiftedSoftplus -log2 into b2'
    w1s = np.concatenate([R * W1, (1.0 - R) * W1], axis=0)      # [512, 256]
    w1v = np.concatenate([R * V1, (1.0 - R) * V1], axis=0)
    b2s = (b2 - LOG2 * W2.sum(axis=0)).astype(np.float32)
    b2v = (-LOG2 * V2.sum(axis=0)).astype(np.float32)

    w1_arr = np.stack([w1s, w1v]).reshape(2, 4, 128, 256).astype(BF)
    w2_arr = np.stack([W2, V2]).reshape(2, 2, 128, 256).astype(BF)
    b2_arr = np.stack([b2s, b2v]).reshape(2, 2, 128).astype(np.float32)
    b1_arr = b1.reshape(2, 128, 1).astype(np.float32)  # [mc, 128, 1]

    ss = np.asarray(inputs["scalar_short"], np.float32)
    sl = np.asarray(inputs["scalar_long"], np.float32)
    vs = np.asarray(inputs["vector_short"], np.float32)
    vl = np.asarray(inputs["vector_long"], np.float32)

    assert counts.max() <= 128, "fragment larger than one block"
    order = np.argsort(frag, kind="stable")
    fstart = np.zeros(NF + 1, np.int64)
    np.cumsum(counts, out=fstart[1:])

    in_maps = []
    meta = []
    for d in range(NCORES):
        g0, g1 = d * NFL, (d + 1) * NFL
        # Bin-pack this device's fragments into 128-node blocks (first-fit
        # decreasing) so no fragment crosses a block boundary: pair tiles
        # then reduce to single [128,128] diagonal blocks.
        fcnt = [(int(counts[g0 + f]), f) for f in range(NFL)]
        fcnt.sort(reverse=True)
        blocks = []  # list of (used, [frag...])
        for c, f in fcnt:
            if c == 0:
                continue
            for blk in blocks:
                if blk[0] + c <= 128:
                    blk[0] += c
                    blk[1].append(f)
                    break
            else:
                blocks.append([c, [f]])
        assert len(blocks) <= NCH, f"device {d}: {len(blocks)} blocks > {NCH}"
        # node order: block by block, fragments whole, pad each block to 128
        nodes = np.zeros(NCAP, np.int64)
        floc = np.full(NCAP, -1, np.int64)
        spans = [None] * NFL  # local frag -> (start, count)
        real = np.zeros(NCAP, bool)
        for bi, (_, fl) in enumerate(blocks):
            pos = bi * 128
            for f in fl:
                g = g0 + f
                c = int(counts[g])
                nodes[pos : pos + c] = order[fstart[g] : fstart[g] + c]
                floc[pos : pos + c] = f
                real[pos : pos + c] = True
                spans[f] = (pos, c)
                pos += c
        nd = NCAP

        def padT(x):  # gather+transpose -> [256, NCAP] bf16, pads zero
            out = np.zeros((256, NCAP), np.float32)
            out[:, real] = x[nodes[real]].T
            return out

        xs = [
            np.concatenate([padT(ss), padT(sl)], axis=0),
            np.concatenate([padT(vs[:, 0, :]), padT(vl[:, 0, :])], axis=0),
            np.concatenate([padT(vs[:, 1, :]), padT(vl[:, 1, :])], axis=0),
            np.concatenate([padT(vs[:, 2, :]), padT(vl[:, 2, :])], axis=0),
        ]
        xin = np.stack(xs).reshape(4, 4, 128, NCAP).astype(BF)

        # block-diagonal mask: tile t = [i in block t, j in block t]
        msk = np.zeros((128, NCH, 128), np.float32)
        for t in range(NCH):
            idxs = 128 * t + np.arange(128)
            fi = floc[idxs]
            msk[:, t, :] = (
                (fi[:, None] >= 0)
                & (fi[None, :] >= 0)
                & (fi[:, None] == fi[None, :])
                & (idxs[None, :] > idxs[:, None])
            )
        msk_arr = msk.reshape(128, NCH * 128).astype(BF)

        oh = np.zeros((NCH, 128, NFL), np.float32)
        idx = np.arange(NCAP)[real]
        oh[idx // 128, idx % 128, floc[idx]] = 1.0
        oh_arr = oh.astype(BF)

        in_maps.append(
            {
                "xin": xin,
                "w1": w1_arr,
                "w2": w2_arr,
                "b2": b2_arr,
                "b1": b1_arr,
                "mskt": msk_arr,
                "oneh": oh_arr,
            }
        )
        meta.append((g0, spans))
    return in_maps, meta, counts


def _host_combine(results, meta, counts):
    total = 0.0
    nvalid = 0
    for d in range(NCORES):
        seg = np.asarray(results[d]["seg"], np.float64)      # [NFL, 1024]
        errc = np.asarray(results[d]["errc"], np.float64)    # [4, 128, NCH]
        nsqo = np.asarray(results[d]["nsqo"], np.float64)    # [4, 128, NCH]
        err_nodes = errc.transpose(0, 2, 1).reshape(4, NCAP)
        nsq_nodes = nsqo.transpose(0, 2, 1).reshape(4, NCAP)
        g0, spans = meta[d]
        # per-local-fragment sums over nodes
        for f in range(NFL):
            c = int(counts[g0 + f])
            if c == 0:
                continue
            pos, c2 = spans[f]
            assert c2 == c
            sl_ = slice(pos, pos + c)
            pc = c * (c - 1) * 0.5
            pcs = max(pc, 1.0)
            ns = float(c)

            xsq_s = nsq_nodes[0, sl_].sum()
            Ssq = (seg[f, 0:256] ** 2).sum()
            s_var = (xsq_s - Ssq / ns) / ns
            s_sim = err_nodes[0, sl_].sum() / pcs
            scalar_loss = s_var + s_sim

            xsq_v = nsq_nodes[1:4, sl_].sum()
            Vsq = (seg[f, 256:1024] ** 2).sum()
            v_var = (xsq_v - Vsq / ns) / ns
            dir_loss = err_nodes[1:4, sl_].sum() / (3.0 * pcs)
            vector_loss = v_var + dir_loss

            if pc > 0:
                total += (1.0 - VW) * scalar_loss + VW * vector_loss
                nvalid += 1
    if nvalid == 0:
        return np.float32(0.0)
    return np.float32(total / nvalid)


TRACE = False
LAST_RESULT = None


def kernel(**inputs):
    global LAST_RESULT
    if "nc" not in _PROG_CACHE:
        _PROG_CACHE["nc"] = _build_program()
    nc = _PROG_CACHE["nc"]

    in_maps, meta, counts = _host_prep(inputs)

    from concourse.bass_utils import run_bass_kernel_spmd

    res = run_bass_kernel_spmd(
        nc, in_maps, core_ids=list(range(NCORES)), trace=TRACE
    )
    LAST_RESULT = res
    return _host_combine(res.results, meta, counts)


if __name__ == "__main__":
    import reference

    inputs = {k: np.asarray(v) for k, v in reference.setup_inputs().items()}
    out = kernel(**inputs)
    print("kernel out:", out)
